# revision 25
# baseline (speedup 1.0000x reference)
"""Causal self-attention on 8 TRN2 NeuronCores.

Sharding: core c handles batch b = c//2 and head-group g = c%2 (8 of 16
heads).  Each core computes qkv for its heads, per-head causal attention,
and a partial output projection (its heads' rows of w_proj).  The two
partial projections per batch are summed on the host (plus b_proj) — no
on-chip collectives.

Per-core layout: everything that feeds the PE array keeps the contraction
dim on partitions.  q^T/k^T are produced directly as [head_dim, token] so
attention scores are computed transposed (att^T[j,i], keys on partitions)
and no PE transposes are needed anywhere.  Softmax is max-free (scores
are O(1) by construction) and the denominators come from a ones-column
appended to V.  The reciprocal runs on DVE over a [128, 4]
partition-scattered copy of the denominators (native InstReciprocal is
per-column, so the scatter makes it 128x cheaper than on the [1, 512]
row), with DMA bounces through DRAM doing the scatter and the final
partition-broadcast.

Causal trimming: diagonal 128-key blocks only stream the unmasked query
range ([128m, 512) within the chunk) in the scores matmul, the exp, and
the AV matmul; the causal mask multiply is a single [128,128] triangle
window per diagonal block.

Scheduling: the attention phase is ACT(exp)-bound, and any PE idle gap
risks a HAM duty-cycle throttle that halves the PE clock for tens of
microseconds.  So attention runs query-chunk-outer (qc, then head), which
legally defers most of the v-projection into the late, expensive chunks,
and all remaining qkv-projection work is drip-fed between score/AV
matmul pairs in ~2-matmul chunks so the PE always has work while ACT
catches up on exp.  A dependency tracker force-completes any unit an
attention step needs first.
"""

import sys

sys.path.insert(0, "/opt/trn_rl_repo")

import ml_dtypes
import numpy as np

import bass_rust
import concourse.bass as bass
import concourse.mybir as mybir
import concourse.tile as tile
from concourse import bass_utils
from concourse.tile import ScopedClock

B, T, C = 4, 2048, 1024
H, HD = 16, 64
HPC = 8  # heads per core
GC = HPC * HD  # 512 cols per head-group
QB = 512  # query chunk (matmul N / PSUM bank limit)
KBLK = 128  # key block (matmul M)
NQC = T // QB  # 4
NKT = T // KBLK  # 16
KT = C // 128  # 8 k-tiles for the qkv projection

F32 = mybir.dt.float32
BF16 = mybir.dt.bfloat16
BF16NP = ml_dtypes.bfloat16


_MAX_WAITS = 1  # walrus in this container rejects >1 sync wait per instruction


def _split_multi_waits(nc: bass.Bass) -> None:
    """Hoist extra sem-waits onto single-wait nops inserted just before the
    owning instruction (same engine), so no instruction carries more than
    _MAX_WAITS waits."""
    eng_by_type = nc.engines

    n_es = [0]

    def make_nop(engine_type, wait):
        # A bare EventSemaphore (what a standalone wait_ge lowers to) — a
        # plain NoOp risks being elided by walrus along with its wait.
        inst = mybir.InstEventSemaphore(
            name=f"I-wsplit-es-{n_es[0]}", ins=[], outs=[]
        )
        n_es[0] += 1
        inst.engine = engine_type
        inst.sync_info = bass_rust.SyncInfo(on_wait=[wait], on_update=[])
        return inst

    for f in nc.m.functions:
        for bb in f.blocks:
            changed = False
            new_insts = []
            for inst in bb.instructions:
                si = inst.sync_info
                waits = list(si.on_wait) if si is not None and si.on_wait else []
                if len(waits) > _MAX_WAITS:
                    for w in waits[:-_MAX_WAITS]:
                        new_insts.append(make_nop(inst.engine, w))
                    si.on_wait = waits[-_MAX_WAITS:]
                    changed = True
                new_insts.append(inst)
            if changed:
                bb.instructions = new_insts


def _dedup_ldweights(nc: bass.Bass) -> None:
    """Drop an InstLdweights when the previous PE weights load was identical
    and only matmuls / event-semaphores ran on the PE in between (the PE
    array still holds those weights)."""
    PE = mybir.EngineType.PE

    def sig(l):
        p = l.ins[0]
        return (
            p.memref,
            p.offset,
            str(p.ap),
            str(p.dtype),
            str(l.perf_mode),
            str(l.is_transpose),
        )

    for f in nc.m.functions:
        for bb in f.blocks:
            prev_sig = None
            new_insts = []
            for inst in bb.instructions:
                if inst.engine != PE:
                    new_insts.append(inst)
                    continue
                if isinstance(inst, mybir.InstLdweights):
                    s = sig(inst)
                    si = inst.sync_info
                    no_sync = si is None or (not si.on_wait and not si.on_update)
                    if s == prev_sig and no_sync:
                        continue  # weights already resident
                    prev_sig = s
                elif not isinstance(
                    inst, (mybir.InstMatmult, mybir.InstEventSemaphore)
                ):
                    prev_sig = None
                new_insts.append(inst)
            bb.instructions = new_insts


def _drain_and_barrier_split(self, tick_clock, wait_clock):
    nc = self.nc
    drain_inst = nc.sync.drain()
    wait_clock.add_sem_waits(
        drain_inst.ins, ScopedClock({None: tick_clock.global_clock})
    )
    nc.all_engine_barrier()
    assert self.sems is not None
    popped = nc._tile_sem_poison_stack.pop()
    assert popped is self._sem_poison
    nc.clear_and_free_semaphores(list(self.sems.allocated().values()))
    nc.all_engine_barrier()
    _dedup_ldweights(nc)
    _split_multi_waits(nc)


tile.TileContext._drain_and_barrier = _drain_and_barrier_split


def build_nc(with_bias: bool) -> bass.Bass:
    nc = bass.Bass("TRN2", target_bir_lowering=False)

    xT = nc.declare_dram_parameter("xT", [C, T], BF16, isOutput=False)
    wqk = nc.declare_dram_parameter("wqk", [C, 2 * GC], BF16, isOutput=False)
    wv = nc.declare_dram_parameter("wv", [C, GC], BF16, isOutput=False)
    wp = nc.declare_dram_parameter("wp", [GC, C], BF16, isOutput=False)
    maskp = nc.declare_dram_parameter("mask", [128, 128], BF16, isOutput=False)
    if with_bias:
        bqk = nc.declare_dram_parameter("bqk", [1, 2 * GC], BF16, isOutput=False)
        bv = nc.declare_dram_parameter("bv", [1, GC], BF16, isOutput=False)
    out = nc.declare_dram_parameter("out", [T, C], BF16, isOutput=True)

    with tile.TileContext(nc) as tc:
        with (
            tc.tile_pool(name="singles", bufs=1) as singles,
            tc.tile_pool(name="exp", bufs=8) as exp_pool,
            tc.tile_pool(name="small", bufs=3) as small_pool,
            tc.tile_pool(name="recipp", bufs=3) as recip_pool,
            tc.tile_pool(name="ytu", bufs=4) as ytu_pool,
            tc.tile_pool(name="outsb", bufs=2) as out_pool,
            tc.tile_pool(name="dram", bufs=8, space="DRAM") as dram_pool,
            tc.tile_pool(name="ps", bufs=2, space="PSUM") as ps_pool,
            tc.tile_pool(name="ps_att", bufs=2, space="PSUM") as ps_att_pool,
            tc.tile_pool(name="ps_y", bufs=2, space="PSUM") as ps_y_pool,
        ):
            # ---- persistent SBUF tensors -------------------------------
            xT_sbs = [
                singles.tile([128, T], BF16, tag=f"xT{kt}", name=f"xT{kt}")
                for kt in range(KT)
            ]
            wqk_sbs = [
                singles.tile([128, 2 * GC], BF16, tag=f"wqk{kt}", name=f"wqk{kt}")
                for kt in range(KT)
            ]
            wv_sbs = [
                singles.tile([128, GC], BF16, tag=f"wv{kt}", name=f"wv{kt}")
                for kt in range(KT)
            ]
            wp_sb = singles.tile([128, 4, C], BF16, tag="wp")
            tri_sb = singles.tile([128, 128], BF16, tag="tri")
            qkT_sbs = [
                singles.tile([128, T], BF16, tag=f"qkT{mt}", name=f"qkT{mt}")
                for mt in range(8)
            ]
            vv_sb = singles.tile([128, HPC, NKT, HD + 1], BF16, tag="vv")
            outA_sbs = [
                singles.tile([128, C], BF16, tag=f"outA{tt}", name=f"outA{tt}")
                for tt in range(NKT)
            ]
            yTn_sbs = [
                singles.tile([128, T], BF16, tag=f"yTn{ct}", name=f"yTn{ct}")
                for ct in range(4)
            ]

            for kt in range(KT):
                qx = nc.scalar if kt % 2 == 0 else nc.sync
                qw = nc.sync if kt % 2 == 0 else nc.scalar
                qx.dma_start(
                    out=xT_sbs[kt][:], in_=xT[kt * 128 : (kt + 1) * 128, :]
                )
                qw.dma_start(
                    out=wqk_sbs[kt][:], in_=wqk[kt * 128 : (kt + 1) * 128, :]
                )
                qw.dma_start(
                    out=wv_sbs[kt][:], in_=wv[kt * 128 : (kt + 1) * 128, :]
                )
            nc.sync.dma_start(
                out=wp_sb[:], in_=wp.rearrange("(ct p) m -> p ct m", p=128)
            )
            nc.sync.dma_start(out=tri_sb[:], in_=maskp[:, :])
            if with_bias:
                bqk_sb = singles.tile([1, 2 * GC], BF16, tag="bqk")
                bv_sb = singles.tile([1, GC], BF16, tag="bv")
                ones_sb = singles.tile([1, T], BF16, tag="ones")
                nc.sync.dma_start(out=bqk_sb[:], in_=bqk[:, :])
                nc.sync.dma_start(out=bv_sb[:], in_=bv[:, :])
                nc.vector.memset(ones_sb[:], 1.0)

            # ones column of v' (the softmax-denominator row of y^T)
            nc.vector.memset(vv_sb[:, :, :, HD], 1.0)

            # ---- filler units: qkv projection work, emitted in ~2-matmul
            # ---- chunks between attention matmuls ----------------------
            def v_unit_steps(tt):
                # v[token 128, col 512] = x @ wv for one token tile
                ps = ps_pool.tile([128, QB], F32, tag="ps", name="ps")
                for kt in range(KT):
                    nc.tensor.matmul(
                        ps[:],
                        lhsT=xT_sbs[kt][:, tt * 128 : (tt + 1) * 128],
                        rhs=wv_sbs[kt][:],
                        start=(kt == 0),
                        stop=(kt == KT - 1 and not with_bias),
                    )
                    if kt % 2 == 1:
                        yield
                if with_bias:
                    nc.tensor.matmul(
                        ps[:],
                        lhsT=ones_sb[0:1, tt * 128 : (tt + 1) * 128],
                        rhs=bv_sb[0:1, :],
                        start=False,
                        stop=True,
                    )
                nc.vector.tensor_copy(
                    vv_sb[:, :, tt, 0:HD],
                    ps[:].rearrange("p (h d) -> p h d", h=HPC),
                )
                yield

            def qk_unit_steps(mt, nta):
                # q^T/k^T [col 128, token 1024] for chunks 2*nta, 2*nta+1;
                # kt-outer with both chunks inner so consecutive matmuls
                # share the same stationary weights
                psA = ps_pool.tile([128, QB], F32, tag="ps", name="psA")
                psB = ps_pool.tile([128, QB], F32, tag="ps", name="psB")
                ca, cb = 2 * nta, 2 * nta + 1
                for kt in range(KT):
                    lw = wqk_sbs[kt][:, mt * 128 : (mt + 1) * 128]
                    nc.tensor.matmul(
                        psA[:],
                        lhsT=lw,
                        rhs=xT_sbs[kt][:, ca * QB : (ca + 1) * QB],
                        start=(kt == 0),
                        stop=(kt == KT - 1 and not with_bias),
                    )
                    nc.tensor.matmul(
                        psB[:],
                        lhsT=lw,
                        rhs=xT_sbs[kt][:, cb * QB : (cb + 1) * QB],
                        start=(kt == 0),
                        stop=(kt == KT - 1 and not with_bias),
                    )
                    yield
                if with_bias:
                    lb = bqk_sb[0:1, mt * 128 : (mt + 1) * 128]
                    nc.tensor.matmul(
                        psA[:],
                        lhsT=lb,
                        rhs=ones_sb[0:1, ca * QB : (ca + 1) * QB],
                        start=False,
                        stop=True,
                    )
                    nc.tensor.matmul(
                        psB[:],
                        lhsT=lb,
                        rhs=ones_sb[0:1, cb * QB : (cb + 1) * QB],
                        start=False,
                        stop=True,
                    )
                nc.vector.tensor_copy(
                    qkT_sbs[mt][:, ca * QB : (ca + 1) * QB], psA[:]
                )
                nc.vector.tensor_copy(
                    qkT_sbs[mt][:, cb * QB : (cb + 1) * QB], psB[:]
                )
                yield

            def proj_a_steps(tt, half):
                # projection wave: head groups 0 and 1, one 512-column
                # slab per unit so consecutive units overlap through the
                # 2-buffer psum pool
                ps = ps_pool.tile([128, QB], F32, tag="ps", name="ps")
                for i, ct in enumerate((0, 1)):
                    nc.tensor.matmul(
                        ps[:],
                        lhsT=yTn_sbs[ct][:, tt * 128 : (tt + 1) * 128],
                        rhs=wp_sb[:, ct, half * QB : (half + 1) * QB],
                        start=(i == 0),
                        stop=(i == 1),
                    )
                yield
                if (tt + half) % 2 == 0:
                    nc.vector.tensor_copy(
                        outA_sbs[tt][:, half * QB : (half + 1) * QB], ps[:]
                    )
                else:
                    # scalar engine can also drain PSUM
                    nc.scalar.activation(
                        outA_sbs[tt][:, half * QB : (half + 1) * QB],
                        ps[:],
                        mybir.ActivationFunctionType.Copy,
                    )
                yield

            def make_steps(unit):
                if unit[0] == "v":
                    return v_unit_steps(unit[1])
                if unit[0] == "pa":
                    return proj_a_steps(unit[1], unit[2])
                return qk_unit_steps(unit[1], unit[2])

            done = set()
            active = [None, None]  # [unit, generator]

            def finish(unit):
                """Run a unit's generator to completion right now."""
                if unit in done:
                    return
                if active[0] == unit:
                    for _ in active[1]:
                        pass
                    active[0] = active[1] = None
                else:
                    for _ in make_steps(unit):
                        pass
                done.add(unit)

            queue = []
            qpos = [0]

            def chunk():
                """Emit ~one chunk (2 matmuls) of filler work."""
                if active[0] is None:
                    while qpos[0] < len(queue):
                        u = queue[qpos[0]]
                        qpos[0] += 1
                        if u not in done:
                            active[0] = u
                            active[1] = make_steps(u)
                            break
                    else:
                        return
                try:
                    next(active[1])
                except StopIteration:
                    done.add(active[0])
                    active[0] = active[1] = None

            def require(units):
                for u in units:
                    if u in done:
                        continue
                    # drain queue entries (fully) up to and including u
                    while u not in done:
                        if active[0] is not None:
                            finish(active[0])
                            continue
                        if qpos[0] < len(queue):
                            nxt = queue[qpos[0]]
                            qpos[0] += 1
                            if nxt not in done:
                                finish(nxt)
                        else:
                            finish(u)

            # prologue: just enough for (qc=0, h=0)
            for u in [("qk", 0, 0), ("qk", 4, 0), ("v", 0), ("v", 1), ("v", 2), ("v", 3)]:
                finish(u)

            # remaining units in first-need order under qc-outer traversal
            queue.extend(
                [("qk", 1, 0), ("qk", 5, 0), ("qk", 2, 0), ("qk", 6, 0),
                 ("qk", 3, 0), ("qk", 7, 0)]
                + [("v", tt) for tt in range(4, 8)]
                + [("qk", 0, 1), ("qk", 4, 1)]
                + [("v", tt) for tt in range(8, 12)]
                + [("qk", 1, 1), ("qk", 5, 1), ("qk", 2, 1), ("qk", 6, 1),
                   ("qk", 3, 1), ("qk", 7, 1)]
                + [("v", tt) for tt in range(12, 16)]
            )

            # ---- attention, transposed: att^T[j, i], qc-outer ----------
            for qc in range(NQC):
                for h in range(HPC):
                    require(
                        [("qk", h // 2, qc // 2)]
                        + [("qk", 4 + h // 2, nta) for nta in range(qc // 2 + 1)]
                        + [("v", tt) for tt in range(4 * qc + 4)]
                    )
                    prt = 64 * (h % 2)
                    qt = qkT_sbs[h // 2]
                    kt_sb = qkT_sbs[4 + h // 2]
                    nkb = 4 * (qc + 1)
                    exp_ts = []
                    # phase A: score matmuls (diagonal blocks trimmed to
                    # the unmasked query range), exp, triangle masks
                    for kb2 in range(0, nkb, 2):
                        ps_att = ps_att_pool.tile([128, 2 * QB], F32, tag="ps_att")
                        m0 = kb2 - 4 * qc
                        for u in (0, 1):
                            kb = kb2 + u
                            m = kb - 4 * qc
                            off = 128 * m if m > 0 else 0
                            nc.tensor.matmul(
                                ps_att[:, u * QB + off : (u + 1) * QB],
                                lhsT=kt_sb[prt : prt + 64, kb * 128 : (kb + 1) * 128],
                                rhs=qt[prt : prt + 64, qc * QB + off : (qc + 1) * QB],
                                start=True,
                                stop=True,
                            )
                        exp_t = exp_pool.tile([128, 2 * QB], BF16, tag="exp")
                        estart = 128 * m0 if m0 > 0 else 0
                        nc.scalar.activation(
                            exp_t[:, estart:],
                            ps_att[:, estart:],
                            mybir.ActivationFunctionType.Exp,
                            scale=0.125,
                        )
                        for u in (0, 1):
                            kb = kb2 + u
                            m = kb - 4 * qc
                            if m >= 0:  # diagonal block: mask the boundary
                                lo = u * QB + 128 * m
                                nc.vector.tensor_mul(
                                    exp_t[:, lo : lo + 128],
                                    exp_t[:, lo : lo + 128],
                                    tri_sb[:],
                                )
                        exp_ts.append(exp_t)
                        chunk()  # keep the PE fed while ACT runs exp
                    # phase B: AV matmuls (diagonal blocks trimmed the same)
                    ps_y = ps_y_pool.tile([HD + 1, QB], F32, tag="ps_y")
                    for kb in range(nkb):
                        m = kb - 4 * qc
                        off = 128 * m if m > 0 else 0
                        nc.tensor.matmul(
                            ps_y[:, off:],
                            lhsT=vv_sb[:, h, kb, :],
                            rhs=exp_ts[kb // 2][
                                :, (kb % 2) * QB + off : (kb % 2 + 1) * QB
                            ],
                            start=(kb == 0),
                            stop=(kb == nkb - 1),
                        )
                        if kb % 2 == 1:
                            chunk()
                    # unnormalized y^T rows 0..63 + denominator row 64
                    ytu = ytu_pool.tile([HD + 1, QB], F32, tag="ytu")
                    nc.vector.tensor_copy(ytu[:], ps_y[:])
                    if qc == NQC - 1 and h >= HPC - 2:
                        # last units: shortest-latency reciprocal on the
                        # (by now idle) scalar engine: 1/x = exp(-ln x)
                        ln_t = recip_pool.tile([1, QB], F32, tag="ln")
                        nc.scalar.activation(
                            ln_t[:],
                            ytu[HD : HD + 1, :],
                            mybir.ActivationFunctionType.Ln,
                        )
                        rec_row = recip_pool.tile([1, QB], F32, tag="rec_row")
                        nc.scalar.activation(
                            rec_row[:],
                            ln_t[:],
                            mybir.ActivationFunctionType.Exp,
                            scale=-1.0,
                        )
                        rec_dram = dram_pool.tile([1, QB], F32, tag="rec_dram")
                        nc.sync.dma_start(out=rec_dram[:], in_=rec_row[:])
                    else:
                        # scatter [1,512] -> [128,4] via a DRAM bounce so
                        # the native per-column DVE reciprocal touches only
                        # 4 columns, then bounce back
                        den_dram = dram_pool.tile([1, QB], F32, tag="den_dram")
                        nc.sync.dma_start(
                            out=den_dram[:], in_=ytu[HD : HD + 1, :]
                        )
                        den_sc = recip_pool.tile([128, 4], F32, tag="den_sc")
                        nc.sync.dma_start(out=den_sc[:], in_=den_dram[:])
                        rec_sc = recip_pool.tile([128, 4], F32, tag="rec_sc")
                        nc.vector.reciprocal(rec_sc[:], den_sc[:])
                        rec_dram = dram_pool.tile([1, QB], F32, tag="rec_dram")
                        nc.sync.dma_start(out=rec_dram[:], in_=rec_sc[:])
                    bcast = small_pool.tile([64, QB], F32, tag="bcast")
                    nc.sync.dma_start(
                        out=bcast[:], in_=rec_dram[:].to_broadcast((64, QB))
                    )
                    nc.vector.tensor_mul(
                        yTn_sbs[h // 2][prt : prt + 64, qc * QB : (qc + 1) * QB],
                        ytu[0:HD, :],
                        bcast[:],
                    )
                    if qc == NQC - 1 and h == 3:
                        # head groups 0,1 fully normalized: the first half
                        # of the projection runs as late-phase filler
                        queue.extend(
                            [("pa", tt, half) for tt in range(NKT) for half in (0, 1)]
                        )

            # flush any unemitted filler units
            while qpos[0] < len(queue) or active[0] is not None:
                chunk()
                if active[0] is None and qpos[0] >= len(queue):
                    break

            # ---- projection, second half + add of the first half -------
            for tt in range(NKT):
                out_sb = out_pool.tile([128, C], BF16, tag="out_sb")
                psw = ps_att_pool.tile([128, 2 * QB], F32, tag="ps_att")
                for half in (0, 1):
                    for ct in (2, 3):
                        nc.tensor.matmul(
                            psw[:, half * QB : (half + 1) * QB],
                            lhsT=yTn_sbs[ct][:, tt * 128 : (tt + 1) * 128],
                            rhs=wp_sb[:, ct, half * QB : (half + 1) * QB],
                            start=(ct == 2),
                            stop=(ct == 3),
                        )
                    nc.vector.tensor_tensor(
                        out_sb[:, half * QB : (half + 1) * QB],
                        psw[:, half * QB : (half + 1) * QB],
                        outA_sbs[tt][:, half * QB : (half + 1) * QB],
                        mybir.AluOpType.add,
                    )
                nc.scalar.dma_start(
                    out=out[tt * 128 : (tt + 1) * 128, :], in_=out_sb[:]
                )

    return nc


def _make_mask() -> np.ndarray:
    # tri[p, j] = 1 iff key p <= query j, the causal window of any
    # diagonal 128x128 block
    p = np.arange(128)[:, None]
    j = np.arange(128)[None, :]
    return (p <= j).astype(BF16NP)


_NC_CACHE: dict[bool, bass.Bass] = {}


def kernel(x, w_qkv, b_qkv, w_proj, b_proj):
    x = np.asarray(x, dtype=np.float32)
    w_qkv = np.asarray(w_qkv, dtype=np.float32)
    b_qkv = np.asarray(b_qkv, dtype=np.float32)
    w_proj = np.asarray(w_proj, dtype=np.float32)
    b_proj = np.asarray(b_proj, dtype=np.float32)

    with_bias = bool(np.any(b_qkv))
    if with_bias not in _NC_CACHE:
        _NC_CACHE[with_bias] = build_nc(with_bias)
    nc = _NC_CACHE[with_bias]

    mask = _make_mask()
    in_maps = []
    for c in range(8):
        b, g = c // 2, c % 2
        cols = slice(g * GC, (g + 1) * GC)
        m = {
            "xT": np.ascontiguousarray(x[b].T).astype(BF16NP),
            "wqk": np.concatenate(
                [w_qkv[:, cols], w_qkv[:, C:][:, cols]], axis=1
            ).astype(BF16NP),
            "wv": np.ascontiguousarray(w_qkv[:, 2 * C :][:, cols]).astype(BF16NP),
            "wp": np.ascontiguousarray(w_proj[cols, :]).astype(BF16NP),
            "mask": mask,
        }
        if with_bias:
            m["bqk"] = np.concatenate([b_qkv[cols], b_qkv[C:][cols]])[None, :].astype(
                BF16NP
            )
            m["bv"] = b_qkv[2 * C :][cols][None, :].astype(BF16NP)
        in_maps.append(m)

    out = np.empty((B, T, C), dtype=np.float32)
    for attempt in range(3):
        res = bass_utils.run_bass_kernel_spmd(nc, in_maps, core_ids=list(range(8)))
        for b in range(B):
            out[b] = (
                res.results[2 * b]["out"].astype(np.float32)
                + res.results[2 * b + 1]["out"].astype(np.float32)
                + b_proj
            )
        if np.isfinite(out).all():
            break
    return out


# revision 27
# speedup vs baseline: 1.1356x; 1.1356x over previous
"""Causal self-attention on 8 TRN2 NeuronCores.

Sharding: core c handles batch b = c//2 and head-group g = c%2 (8 of 16
heads).  Each core computes qkv for its heads, per-head causal attention,
and a partial output projection (its heads' rows of w_proj).  The two
partial projections per batch are summed on the host (plus b_proj) — no
on-chip collectives.

Per-core layout: everything that feeds the PE array keeps the contraction
dim on partitions.  q^T/k^T are produced directly as [head_dim, token] so
attention scores are computed transposed (att^T[j,i], keys on partitions)
and no PE transposes are needed anywhere.  Softmax is max-free (scores
are O(1) by construction) and the denominators come from a ones-column
appended to V.  The reciprocal runs on DVE over a [128, 4]
partition-scattered copy of the denominators (native InstReciprocal is
per-column, so the scatter makes it 128x cheaper than on the [1, 512]
row), with DMA bounces through DRAM doing the scatter and the final
partition-broadcast.

Causal trimming: diagonal 128-key blocks only stream the unmasked query
range ([128m, 512) within the chunk) in the scores matmul, the exp, and
the AV matmul; the causal mask multiply is a single [128,128] triangle
window per diagonal block.

Scheduling: the attention phase is ACT(exp)-bound, and any PE idle gap
risks a HAM duty-cycle throttle that halves the PE clock for tens of
microseconds.  So attention runs query-chunk-outer (qc, then head), which
legally defers most of the v-projection into the late, expensive chunks,
and all remaining qkv-projection work is drip-fed between score/AV
matmul pairs in ~2-matmul chunks so the PE always has work while ACT
catches up on exp.  A dependency tracker force-completes any unit an
attention step needs first.
"""

import sys

sys.path.insert(0, "/opt/trn_rl_repo")

import ml_dtypes
import numpy as np

import bass_rust
import concourse.bass as bass
import concourse.mybir as mybir
import concourse.tile as tile
from concourse import bass_utils
from concourse.tile import ScopedClock

B, T, C = 4, 2048, 1024
H, HD = 16, 64
HPC = 8  # heads per core
GC = HPC * HD  # 512 cols per head-group
QB = 512  # query chunk (matmul N / PSUM bank limit)
KBLK = 128  # key block (matmul M)
NQC = T // QB  # 4
NKT = T // KBLK  # 16
KT = C // 128  # 8 k-tiles for the qkv projection

F32 = mybir.dt.float32
BF16 = mybir.dt.bfloat16
BF16NP = ml_dtypes.bfloat16


_MAX_WAITS = 1  # walrus in this container rejects >1 sync wait per instruction


def _split_multi_waits(nc: bass.Bass) -> None:
    """Hoist extra sem-waits onto single-wait nops inserted just before the
    owning instruction (same engine), so no instruction carries more than
    _MAX_WAITS waits."""
    eng_by_type = nc.engines

    n_es = [0]

    def make_nop(engine_type, wait):
        # A bare EventSemaphore (what a standalone wait_ge lowers to) — a
        # plain NoOp risks being elided by walrus along with its wait.
        inst = mybir.InstEventSemaphore(
            name=f"I-wsplit-es-{n_es[0]}", ins=[], outs=[]
        )
        n_es[0] += 1
        inst.engine = engine_type
        inst.sync_info = bass_rust.SyncInfo(on_wait=[wait], on_update=[])
        return inst

    for f in nc.m.functions:
        for bb in f.blocks:
            changed = False
            new_insts = []
            for inst in bb.instructions:
                si = inst.sync_info
                waits = list(si.on_wait) if si is not None and si.on_wait else []
                if len(waits) > _MAX_WAITS:
                    for w in waits[:-_MAX_WAITS]:
                        new_insts.append(make_nop(inst.engine, w))
                    si.on_wait = waits[-_MAX_WAITS:]
                    changed = True
                new_insts.append(inst)
            if changed:
                bb.instructions = new_insts


def _dedup_ldweights(nc: bass.Bass) -> None:
    """Drop an InstLdweights when the previous PE weights load was identical
    and only matmuls / event-semaphores ran on the PE in between (the PE
    array still holds those weights)."""
    PE = mybir.EngineType.PE

    def sig(l):
        p = l.ins[0]
        return (
            p.memref,
            p.offset,
            str(p.ap),
            str(p.dtype),
            str(l.perf_mode),
            str(l.is_transpose),
        )

    for f in nc.m.functions:
        for bb in f.blocks:
            prev_sig = None
            new_insts = []
            for inst in bb.instructions:
                if inst.engine != PE:
                    new_insts.append(inst)
                    continue
                if isinstance(inst, mybir.InstLdweights):
                    s = sig(inst)
                    si = inst.sync_info
                    no_sync = si is None or (not si.on_wait and not si.on_update)
                    if s == prev_sig and no_sync:
                        continue  # weights already resident
                    prev_sig = s
                elif not isinstance(
                    inst, (mybir.InstMatmult, mybir.InstEventSemaphore)
                ):
                    prev_sig = None
                new_insts.append(inst)
            bb.instructions = new_insts


def _drain_and_barrier_split(self, tick_clock, wait_clock):
    nc = self.nc
    drain_inst = nc.sync.drain()
    wait_clock.add_sem_waits(
        drain_inst.ins, ScopedClock({None: tick_clock.global_clock})
    )
    nc.all_engine_barrier()
    assert self.sems is not None
    popped = nc._tile_sem_poison_stack.pop()
    assert popped is self._sem_poison
    nc.clear_and_free_semaphores(list(self.sems.allocated().values()))
    nc.all_engine_barrier()
    _dedup_ldweights(nc)
    _split_multi_waits(nc)


tile.TileContext._drain_and_barrier = _drain_and_barrier_split


def build_nc(with_bias: bool) -> bass.Bass:
    nc = bass.Bass("TRN2", target_bir_lowering=False)

    xT = nc.declare_dram_parameter("xT", [C, T], BF16, isOutput=False)
    wqk = nc.declare_dram_parameter("wqk", [C, 2 * GC], BF16, isOutput=False)
    wv = nc.declare_dram_parameter("wv", [C, GC], BF16, isOutput=False)
    wp = nc.declare_dram_parameter("wp", [GC, C], BF16, isOutput=False)
    maskp = nc.declare_dram_parameter("mask", [128, 256], BF16, isOutput=False)
    if with_bias:
        bqk = nc.declare_dram_parameter("bqk", [1, 2 * GC], BF16, isOutput=False)
        bv = nc.declare_dram_parameter("bv", [1, GC], BF16, isOutput=False)
    out = nc.declare_dram_parameter("out", [T, C], BF16, isOutput=True)

    with tile.TileContext(nc) as tc:
        with (
            tc.tile_pool(name="singles", bufs=1) as singles,
            tc.tile_pool(name="exp", bufs=8) as exp_pool,
            tc.tile_pool(name="small", bufs=3) as small_pool,
            tc.tile_pool(name="recipp", bufs=3) as recip_pool,
            tc.tile_pool(name="ytu", bufs=4) as ytu_pool,
            tc.tile_pool(name="outsb", bufs=2) as out_pool,
            tc.tile_pool(name="dram", bufs=8, space="DRAM") as dram_pool,
            tc.tile_pool(name="ps", bufs=2, space="PSUM") as ps_pool,
            tc.tile_pool(name="ps_att", bufs=2, space="PSUM") as ps_att_pool,
            tc.tile_pool(name="ps_y", bufs=2, space="PSUM") as ps_y_pool,
        ):
            # ---- persistent SBUF tensors -------------------------------
            xT_sbs = [
                singles.tile([128, T], BF16, tag=f"xT{kt}", name=f"xT{kt}")
                for kt in range(KT)
            ]
            wqk_sbs = [
                singles.tile([128, 2 * GC], BF16, tag=f"wqk{kt}", name=f"wqk{kt}")
                for kt in range(KT)
            ]
            wv_sbs = [
                singles.tile([128, GC], BF16, tag=f"wv{kt}", name=f"wv{kt}")
                for kt in range(KT)
            ]
            wp_sb = singles.tile([128, 4, C], BF16, tag="wp")
            tri_sb = singles.tile([128, 256], BF16, tag="tri")
            qkT_sbs = [
                singles.tile([128, T], BF16, tag=f"qkT{mt}", name=f"qkT{mt}")
                for mt in range(8)
            ]
            vv_sb = singles.tile([128, HPC, NKT, HD + 1], BF16, tag="vv")
            outA_sbs = [
                singles.tile([128, C], BF16, tag=f"outA{tt}", name=f"outA{tt}")
                for tt in range(NKT)
            ]
            yTn_sbs = [
                singles.tile([128, T], BF16, tag=f"yTn{ct}", name=f"yTn{ct}")
                for ct in range(4)
            ]

            for kt in range(KT):
                nc.scalar.dma_start(
                    out=xT_sbs[kt][:], in_=xT[kt * 128 : (kt + 1) * 128, :]
                )
                nc.sync.dma_start(
                    out=wqk_sbs[kt][:], in_=wqk[kt * 128 : (kt + 1) * 128, :]
                )
                nc.sync.dma_start(
                    out=wv_sbs[kt][:], in_=wv[kt * 128 : (kt + 1) * 128, :]
                )
            nc.sync.dma_start(
                out=wp_sb[:], in_=wp.rearrange("(ct p) m -> p ct m", p=128)
            )
            nc.sync.dma_start(out=tri_sb[:], in_=maskp[:, :])
            if with_bias:
                bqk_sb = singles.tile([1, 2 * GC], BF16, tag="bqk")
                bv_sb = singles.tile([1, GC], BF16, tag="bv")
                ones_sb = singles.tile([1, T], BF16, tag="ones")
                nc.sync.dma_start(out=bqk_sb[:], in_=bqk[:, :])
                nc.sync.dma_start(out=bv_sb[:], in_=bv[:, :])
                nc.vector.memset(ones_sb[:], 1.0)

            # ones column of v' (the softmax-denominator row of y^T)
            nc.vector.memset(vv_sb[:, :, :, HD], 1.0)

            # ---- filler units: qkv projection work, emitted in ~2-matmul
            # ---- chunks between attention matmuls ----------------------
            def v_unit_steps(tt):
                # v[token 128, col 512] = x @ wv for one token tile
                ps = ps_pool.tile([128, QB], F32, tag="ps", name="ps")
                for kt in range(KT):
                    nc.tensor.matmul(
                        ps[:],
                        lhsT=xT_sbs[kt][:, tt * 128 : (tt + 1) * 128],
                        rhs=wv_sbs[kt][:],
                        start=(kt == 0),
                        stop=(kt == KT - 1 and not with_bias),
                    )
                    if kt % 2 == 1:
                        yield
                if with_bias:
                    nc.tensor.matmul(
                        ps[:],
                        lhsT=ones_sb[0:1, tt * 128 : (tt + 1) * 128],
                        rhs=bv_sb[0:1, :],
                        start=False,
                        stop=True,
                    )
                nc.vector.tensor_copy(
                    vv_sb[:, :, tt, 0:HD],
                    ps[:].rearrange("p (h d) -> p h d", h=HPC),
                )
                yield

            def qk_unit_steps(mt, nta):
                # q^T/k^T [col 128, token 1024] for chunks 2*nta, 2*nta+1;
                # kt-outer with both chunks inner so consecutive matmuls
                # share the same stationary weights
                psA = ps_pool.tile([128, QB], F32, tag="ps", name="psA")
                psB = ps_pool.tile([128, QB], F32, tag="ps", name="psB")
                ca, cb = 2 * nta, 2 * nta + 1
                for kt in range(KT):
                    lw = wqk_sbs[kt][:, mt * 128 : (mt + 1) * 128]
                    nc.tensor.matmul(
                        psA[:],
                        lhsT=lw,
                        rhs=xT_sbs[kt][:, ca * QB : (ca + 1) * QB],
                        start=(kt == 0),
                        stop=(kt == KT - 1 and not with_bias),
                    )
                    nc.tensor.matmul(
                        psB[:],
                        lhsT=lw,
                        rhs=xT_sbs[kt][:, cb * QB : (cb + 1) * QB],
                        start=(kt == 0),
                        stop=(kt == KT - 1 and not with_bias),
                    )
                    yield
                if with_bias:
                    lb = bqk_sb[0:1, mt * 128 : (mt + 1) * 128]
                    nc.tensor.matmul(
                        psA[:],
                        lhsT=lb,
                        rhs=ones_sb[0:1, ca * QB : (ca + 1) * QB],
                        start=False,
                        stop=True,
                    )
                    nc.tensor.matmul(
                        psB[:],
                        lhsT=lb,
                        rhs=ones_sb[0:1, cb * QB : (cb + 1) * QB],
                        start=False,
                        stop=True,
                    )
                nc.vector.tensor_copy(
                    qkT_sbs[mt][:, ca * QB : (ca + 1) * QB], psA[:]
                )
                nc.vector.tensor_copy(
                    qkT_sbs[mt][:, cb * QB : (cb + 1) * QB], psB[:]
                )
                yield

            def proj_a_steps(tt, half):
                # projection wave: head groups 0 and 1, one 512-column
                # slab per unit so consecutive units overlap through the
                # 2-buffer psum pool
                ps = ps_pool.tile([128, QB], F32, tag="ps", name="ps")
                for i, ct in enumerate((0, 1)):
                    nc.tensor.matmul(
                        ps[:],
                        lhsT=yTn_sbs[ct][:, tt * 128 : (tt + 1) * 128],
                        rhs=wp_sb[:, ct, half * QB : (half + 1) * QB],
                        start=(i == 0),
                        stop=(i == 1),
                    )
                yield
                if (tt + half) % 2 == 0:
                    nc.vector.tensor_copy(
                        outA_sbs[tt][:, half * QB : (half + 1) * QB], ps[:]
                    )
                else:
                    # scalar engine can also drain PSUM
                    nc.scalar.activation(
                        outA_sbs[tt][:, half * QB : (half + 1) * QB],
                        ps[:],
                        mybir.ActivationFunctionType.Copy,
                    )
                yield

            def make_steps(unit):
                if unit[0] == "v":
                    return v_unit_steps(unit[1])
                if unit[0] == "pa":
                    return proj_a_steps(unit[1], unit[2])
                return qk_unit_steps(unit[1], unit[2])

            done = set()
            active = [None, None]  # [unit, generator]

            def finish(unit):
                """Run a unit's generator to completion right now."""
                if unit in done:
                    return
                if active[0] == unit:
                    for _ in active[1]:
                        pass
                    active[0] = active[1] = None
                else:
                    for _ in make_steps(unit):
                        pass
                done.add(unit)

            queue = []
            qpos = [0]

            def chunk():
                """Emit ~one chunk (2 matmuls) of filler work."""
                if active[0] is None:
                    while qpos[0] < len(queue):
                        u = queue[qpos[0]]
                        qpos[0] += 1
                        if u not in done:
                            active[0] = u
                            active[1] = make_steps(u)
                            break
                    else:
                        return
                try:
                    next(active[1])
                except StopIteration:
                    done.add(active[0])
                    active[0] = active[1] = None

            def require(units):
                for u in units:
                    if u in done:
                        continue
                    # drain queue entries (fully) up to and including u
                    while u not in done:
                        if active[0] is not None:
                            finish(active[0])
                            continue
                        if qpos[0] < len(queue):
                            nxt = queue[qpos[0]]
                            qpos[0] += 1
                            if nxt not in done:
                                finish(nxt)
                        else:
                            finish(u)

            # prologue: just enough for (qc=0, h=0)
            for u in [("qk", 0, 0), ("qk", 4, 0), ("v", 0), ("v", 1), ("v", 2), ("v", 3)]:
                finish(u)

            # remaining units in first-need order under qc-outer traversal
            queue.extend(
                [("qk", 1, 0), ("qk", 5, 0), ("qk", 2, 0), ("qk", 6, 0),
                 ("qk", 3, 0), ("qk", 7, 0)]
                + [("v", tt) for tt in range(4, 8)]
                + [("qk", 0, 1), ("qk", 4, 1)]
                + [("v", tt) for tt in range(8, 12)]
                + [("qk", 1, 1), ("qk", 5, 1), ("qk", 2, 1), ("qk", 6, 1),
                   ("qk", 3, 1), ("qk", 7, 1)]
                + [("v", tt) for tt in range(12, 16)]
            )

            # ---- attention, transposed: att^T[j, i], qc-outer ----------
            for qc in range(NQC):
                for h in range(HPC):
                    require(
                        [("qk", h // 2, qc // 2)]
                        + [("qk", 4 + h // 2, nta) for nta in range(qc // 2 + 1)]
                        + [("v", tt) for tt in range(4 * qc + 4)]
                    )
                    prt = 64 * (h % 2)
                    qt = qkT_sbs[h // 2]
                    kt_sb = qkT_sbs[4 + h // 2]
                    nkb = 4 * (qc + 1)
                    exp_ts = []
                    # phase A: score matmuls (diagonal blocks trimmed to
                    # the unmasked query range), exp, triangle masks
                    for kb2 in range(0, nkb, 2):
                        ps_att = ps_att_pool.tile([128, 2 * QB], F32, tag="ps_att")
                        m0 = kb2 - 4 * qc
                        for u in (0, 1):
                            kb = kb2 + u
                            m = kb - 4 * qc
                            off = 128 * m if m > 0 else 0
                            nc.tensor.matmul(
                                ps_att[:, u * QB + off : (u + 1) * QB],
                                lhsT=kt_sb[prt : prt + 64, kb * 128 : (kb + 1) * 128],
                                rhs=qt[prt : prt + 64, qc * QB + off : (qc + 1) * QB],
                                start=True,
                                stop=True,
                            )
                        exp_t = exp_pool.tile([128, 2 * QB], BF16, tag="exp")
                        estart = 128 * m0 if m0 > 0 else 0
                        nc.scalar.activation(
                            exp_t[:, estart:],
                            ps_att[:, estart:],
                            mybir.ActivationFunctionType.Exp,
                            scale=0.125,
                        )
                        for u in (0, 1):
                            kb = kb2 + u
                            m = kb - 4 * qc
                            if m >= 0:  # diagonal block: mask the boundary
                                lo = u * QB + 128 * m
                                nc.vector.tensor_mul(
                                    exp_t[:, lo : lo + 128],
                                    exp_t[:, lo : lo + 128],
                                    tri_sb[:, 0:128],
                                )
                        exp_ts.append(exp_t)
                        chunk()  # keep the PE fed while ACT runs exp
                    # phase B: AV matmuls (diagonal blocks trimmed the same)
                    ps_y = ps_y_pool.tile([HD + 1, QB], F32, tag="ps_y")
                    for kb in range(nkb):
                        m = kb - 4 * qc
                        off = 128 * m if m > 0 else 0
                        nc.tensor.matmul(
                            ps_y[:, off:],
                            lhsT=vv_sb[:, h, kb, :],
                            rhs=exp_ts[kb // 2][
                                :, (kb % 2) * QB + off : (kb % 2 + 1) * QB
                            ],
                            start=(kb == 0),
                            stop=(kb == nkb - 1),
                        )
                        if kb % 2 == 1:
                            chunk()
                    # unnormalized y^T rows 0..63 + denominator row 64
                    ytu = ytu_pool.tile([HD + 1, QB], F32, tag="ytu")
                    nc.vector.tensor_copy(ytu[:], ps_y[:])
                    if qc == NQC - 1 and h >= HPC - 2:
                        # last units: shortest-latency reciprocal on the
                        # (by now idle) scalar engine: 1/x = exp(-ln x)
                        ln_t = recip_pool.tile([1, QB], F32, tag="ln")
                        nc.scalar.activation(
                            ln_t[:],
                            ytu[HD : HD + 1, :],
                            mybir.ActivationFunctionType.Ln,
                        )
                        rec_row = recip_pool.tile([1, QB], F32, tag="rec_row")
                        nc.scalar.activation(
                            rec_row[:],
                            ln_t[:],
                            mybir.ActivationFunctionType.Exp,
                            scale=-1.0,
                        )
                        rec_dram = dram_pool.tile([1, QB], F32, tag="rec_dram")
                        nc.sync.dma_start(out=rec_dram[:], in_=rec_row[:])
                    else:
                        # scatter [1,512] -> [128,4] via a DRAM bounce so
                        # the native per-column DVE reciprocal touches only
                        # 4 columns, then bounce back
                        den_dram = dram_pool.tile([1, QB], F32, tag="den_dram")
                        nc.sync.dma_start(
                            out=den_dram[:], in_=ytu[HD : HD + 1, :]
                        )
                        den_sc = recip_pool.tile([128, 4], F32, tag="den_sc")
                        nc.sync.dma_start(out=den_sc[:], in_=den_dram[:])
                        rec_sc = recip_pool.tile([128, 4], F32, tag="rec_sc")
                        nc.vector.reciprocal(rec_sc[:], den_sc[:])
                        rec_dram = dram_pool.tile([1, QB], F32, tag="rec_dram")
                        nc.sync.dma_start(out=rec_dram[:], in_=rec_sc[:])
                    bcast = small_pool.tile([64, QB], F32, tag="bcast")
                    nc.sync.dma_start(
                        out=bcast[:], in_=rec_dram[:].to_broadcast((64, QB))
                    )
                    nc.vector.tensor_mul(
                        yTn_sbs[h // 2][prt : prt + 64, qc * QB : (qc + 1) * QB],
                        ytu[0:HD, :],
                        bcast[:],
                    )
                    if qc == NQC - 1 and h == 3:
                        # head groups 0,1 fully normalized: the first half
                        # of the projection runs as late-phase filler
                        queue.extend(
                            [("pa", tt, half) for tt in range(NKT) for half in (0, 1)]
                        )

            # flush any unemitted filler units
            while qpos[0] < len(queue) or active[0] is not None:
                chunk()
                if active[0] is None and qpos[0] >= len(queue):
                    break

            # ---- projection, second half + add of the first half -------
            for tt in range(NKT):
                out_sb = out_pool.tile([128, C], BF16, tag="out_sb")
                for half in (0, 1):
                    ps = ps_pool.tile([128, QB], F32, tag="ps", name="ps")
                    for ct in (2, 3):
                        nc.tensor.matmul(
                            ps[:],
                            lhsT=yTn_sbs[ct][:, tt * 128 : (tt + 1) * 128],
                            rhs=wp_sb[:, ct, half * QB : (half + 1) * QB],
                            start=(ct == 2),
                            stop=False,
                        )
                    # add the first projection half via identity matmul so
                    # the sum lands in PSUM and no vector add is needed
                    nc.tensor.matmul(
                        ps[:],
                        lhsT=tri_sb[:, 128:256],
                        rhs=outA_sbs[tt][:, half * QB : (half + 1) * QB],
                        start=False,
                        stop=True,
                    )
                    if (tt + half) % 2 == 0:
                        nc.vector.tensor_copy(
                            out_sb[:, half * QB : (half + 1) * QB], ps[:]
                        )
                    else:
                        nc.scalar.activation(
                            out_sb[:, half * QB : (half + 1) * QB],
                            ps[:],
                            mybir.ActivationFunctionType.Copy,
                        )
                nc.scalar.dma_start(
                    out=out[tt * 128 : (tt + 1) * 128, :], in_=out_sb[:]
                )

    return nc


def _make_mask() -> np.ndarray:
    # cols 0:128 — tri[p, j] = 1 iff key p <= query j, the causal window
    # of any diagonal 128x128 block; cols 128:256 — identity, used as
    # stationary weights to add SBUF tensors into PSUM accumulators
    p = np.arange(128)[:, None]
    j = np.arange(128)[None, :]
    tri = (p <= j).astype(BF16NP)
    return np.concatenate([tri, np.eye(128, dtype=BF16NP)], axis=1)


_NC_CACHE: dict[bool, bass.Bass] = {}


def kernel(x, w_qkv, b_qkv, w_proj, b_proj):
    x = np.asarray(x, dtype=np.float32)
    w_qkv = np.asarray(w_qkv, dtype=np.float32)
    b_qkv = np.asarray(b_qkv, dtype=np.float32)
    w_proj = np.asarray(w_proj, dtype=np.float32)
    b_proj = np.asarray(b_proj, dtype=np.float32)

    with_bias = bool(np.any(b_qkv))
    if with_bias not in _NC_CACHE:
        _NC_CACHE[with_bias] = build_nc(with_bias)
    nc = _NC_CACHE[with_bias]

    mask = _make_mask()
    in_maps = []
    for c in range(8):
        b, g = c // 2, c % 2
        cols = slice(g * GC, (g + 1) * GC)
        m = {
            "xT": np.ascontiguousarray(x[b].T).astype(BF16NP),
            "wqk": np.concatenate(
                [w_qkv[:, cols], w_qkv[:, C:][:, cols]], axis=1
            ).astype(BF16NP),
            "wv": np.ascontiguousarray(w_qkv[:, 2 * C :][:, cols]).astype(BF16NP),
            "wp": np.ascontiguousarray(w_proj[cols, :]).astype(BF16NP),
            "mask": mask,
        }
        if with_bias:
            m["bqk"] = np.concatenate([b_qkv[cols], b_qkv[C:][cols]])[None, :].astype(
                BF16NP
            )
            m["bv"] = b_qkv[2 * C :][cols][None, :].astype(BF16NP)
        in_maps.append(m)

    out = np.empty((B, T, C), dtype=np.float32)
    for attempt in range(3):
        res = bass_utils.run_bass_kernel_spmd(nc, in_maps, core_ids=list(range(8)))
        for b in range(B):
            out[b] = (
                res.results[2 * b]["out"].astype(np.float32)
                + res.results[2 * b + 1]["out"].astype(np.float32)
                + b_proj
            )
        if np.isfinite(out).all():
            break
    return out
